# revision 32
# baseline (speedup 1.0000x reference)
"""Self-contained Trainium2 kernel for nn_DynamicConv2D (moe_routing).

Contract: kernel(**inputs) takes FULL unsharded inputs (numpy), returns the
FULL output [32, 64, 64, 128] float32. Internally shards batch across 8
NeuronCores (4 samples each), runs a Bass/Tile kernel via
run_bass_kernel_spmd, and gathers.

Device-side work per sample:
  pool  = sum(x) over H,W            (piecewise partial reduces on ACT+DVE
                                      chasing the input DMA; 1/4096 folded
                                      into R on host)
  att   = softmax(relu(pool@R')@A')  (tiny PE matmuls + ACT relu/exp + DVE recip)
  wmix  = sum_k att[k] * bank[k]     (DVE scalar_tensor_tensor MACs, fp16,
                                      emitted in 3 tap-groups so the conv can
                                      start after the first group is mixed)
  conv  = 9-tap shifted fp16 matmuls accumulated in PSUM, per 512-pos chunk
  out   = Relu(conv + beta)          (ACT epilogue, per-partition bias;
                                      BN scale folded into bank/bias on host;
                                      fp16 output, host upconverts)

Layout: x is host-transposed to channel-major [C, H, W], zero-padded to
[C, 66, 66], and cast to fp16 so all 9 conv taps are plain access-pattern
offsets; output is produced channel-major [F, H*W] fp16 and host-transposed
back to NHWC. Expert bank is BN-folded, fp16, tap-group-major, replicated
per core.
"""

import os
import sys

if "/opt/trn_rl_repo" not in sys.path:
    sys.path.insert(0, "/opt/trn_rl_repo")
# The kernel executes through the axon PJRT backend; make sure jax can see it
# if the caller's environment doesn't pin a platform.
if not os.environ.get("JAX_PLATFORMS"):
    os.environ["JAX_PLATFORMS"] = "axon"

import numpy as np

import concourse.bacc as bacc
import concourse.tile as tile
from concourse import mybir
from concourse.bass_utils import run_bass_kernel_spmd
from concourse.tile_rust import add_dep_helper


def _ensure_ntff_hook():
    """run_bass_kernel_spmd(trace=True) under axon needs antenv.axon_hooks,
    which this image's antenv package lacks. Register an equivalent module
    (ctypes into libaxon_pjrt.so) so profiled runs work."""
    try:
        from antenv import axon_hooks  # noqa: F401
        return
    except ImportError:
        pass
    import contextlib
    import ctypes
    import os
    import types

    so_path = os.environ.get("AXON_PJRT_SO", "/opt/axon/libaxon_pjrt.so")
    mod = types.ModuleType("antenv.axon_hooks")
    state = {"hook": None}

    def _make_hook():
        if not os.path.exists(so_path):
            return None
        lib = ctypes.CDLL(so_path)
        if not hasattr(lib, "axon_start_nrt_profile"):
            return None
        lib.axon_start_nrt_profile.argtypes = [
            ctypes.POINTER(ctypes.c_int64), ctypes.c_size_t]
        lib.axon_start_nrt_profile.restype = ctypes.c_int64
        lib.axon_stop_nrt_profile.argtypes = [ctypes.c_char_p]
        lib.axon_stop_nrt_profile.restype = ctypes.c_int64

        @contextlib.contextmanager
        def _hook(output_dir, device_ids):
            import jax
            jax.devices()
            if device_ids:
                ids = (ctypes.c_int64 * len(device_ids))(*device_ids)
                rc = lib.axon_start_nrt_profile(ids, len(device_ids))
            else:
                rc = lib.axon_start_nrt_profile(None, 0)
            if rc != 0:
                raise RuntimeError(f"axon_start_nrt_profile rc={rc}")
            try:
                yield
            finally:
                n = lib.axon_stop_nrt_profile(str(output_dir).encode())
                if n < 0:
                    raise RuntimeError(f"axon_stop_nrt_profile rc={n}")

        return _hook

    def get_axon_ntff_profile_hook():
        if state["hook"] is None:
            state["hook"] = _make_hook()
        return state["hook"]

    def set_axon_ntff_profile_hook(hook):
        state["hook"] = hook

    mod.get_axon_ntff_profile_hook = get_axon_ntff_profile_hook
    mod.set_axon_ntff_profile_hook = set_axon_ntff_profile_hook
    sys.modules["antenv.axon_hooks"] = mod
    try:
        import antenv
        antenv.axon_hooks = mod
    except ImportError:
        pass


F32 = mybir.dt.float32
F16 = mybir.dt.float16
AF = mybir.ActivationFunctionType
ALU = mybir.AluOpType

B, H, W, C = 32, 64, 64, 128
NCORES = 8
BPC = B // NCORES  # samples per core
HP, WP = H + 2, W + 2  # zero-padded
NPAD = HP * WP  # 4356
NPOS = H * W  # 4096
K = 4  # experts
NF = 128  # output filters
TAPS = 9
ROWS_PER_CHUNK = 8  # 8 image rows * 64 cols = 512 positions per PSUM chunk
NCHUNK = H // ROWS_PER_CHUNK
HALF = NPAD // 2
GROUPS = 3  # mixing tap-groups
GW = 3 * NF  # 384: wm cols per group
WGK = K * GW  # 1536: wk cols per tap-group (group-major bank layout)
# Consts ride INSIDE sample 0's input tensor as fp16 columns appended
# after the padded image (every separate [128, N] DMA costs ~4.5us in
# row-packet overhead regardless of N, so zero extra transfers):
#   cols NPAD+0..3    red   = reduction_kernel (RAW; 1/4096 moves into the
#                             exp activation's scale argument)
#   cols NPAD+4..7    attk  = attention_kernel / 30     (rows 0-3)
#   cols NPAD+8..135  biasw = bias * inv                (rows 0-3)
#   col  NPAD+136     c1    = bn_bias - bn_mean*inv
XCONST = 137
XCOLS = NPAD + XCONST + 3  # 4496, rounded for alignment
POOL_SPLIT = 2178  # ACT reduces [0:split], DVE reduces [split:NPAD]

# tunables: warm-up matmul counts (keep PE busy/clock-ramped through startup)
WARM1 = 17   # 512-col fp16 warm-ups before chain-0 tiny matmuls
WARM2 = 2    # 256-col warm-ups between chain-0 steps (covers engine hops)
WARM3 = 12   # 512-col warm-ups covering the mixing-group-0 window


class _Consts:
    """AP views into sample 0's appended fp16 constant columns."""

    def __init__(self, x0, ones16, ones32):
        self.red = x0[:, NPAD + 0:NPAD + 4]        # [128, 4] fp16
        self.attk = x0[0:4, NPAD + 4:NPAD + 8]     # [4, 4]   fp16
        self.biasw = x0[0:4, NPAD + 8:NPAD + 136]  # [4, 128] fp16
        self.c1 = x0[:, NPAD + 136:NPAD + 137]     # [128, 1] fp16
        self.ones16 = ones16                       # [1, 128] fp16 memset
        self.ones32 = ones32                       # [1, 128] f32 memset


def _emit_pool(nc, b, sb, xt_sb, trash, act_only=False):
    """Pool half-reduces (ACT + DVE) over the image columns only. act_only
    puts both halves on the scalar engine — used for the last samples so
    their reduces take no DVE time away from the mixing chains. The f32
    partials are converted to fp16 so the pr matmul can consume them
    against the fp16 reduction weights."""
    pq = sb.tile([C, 2], F32, tag="poolh", name=f"pool{b}h")
    ia = nc.scalar.activation(trash[:, :HALF], xt_sb[:, :HALF], AF.Identity,
                              accum_out=pq[:, 0:1])
    if act_only:
        ib = nc.scalar.activation(trash[:, :HALF], xt_sb[:, HALF:NPAD],
                                  AF.Identity, accum_out=pq[:, 1:2])
    else:
        ib = nc.vector.tensor_reduce(pq[:, 1:2], xt_sb[:, HALF:NPAD],
                                     axis=mybir.AxisListType.X, op=ALU.add)
    pq16 = sb.tile([C, 2], F16, tag="poolh16", name=f"pool{b}h16")
    nc.vector.tensor_copy(pq16[:], pq[:])
    return {"pool_a": ia, "pool_b": ib,
            "pq": [pq16[:, 0:1], pq16[:, 1:2]], "act_only": act_only}


def _emit_chain_stage1(nc, sb, ps, cc, pool, warm=None):
    """pr(PE, accumulating over the pool partials) -> relu(ACT)."""
    pq = pool["pq"]
    pr_ps = ps.tile([K, 1], F32, tag="tiny")
    n = len(pq)
    for i in range(n):
        nc.tensor.matmul(pr_ps[:], cc.red, pq[i], start=(i == 0),
                         stop=(i == n - 1))
    if warm:
        warm()
    prelu_sb = sb.tile([K, 1], F16, tag="prelu")
    nc.scalar.activation(prelu_sb[:], pr_ps[:], AF.Relu)
    return {"prelu": prelu_sb}


def _emit_chain_stage2(nc, sb, ps, cc, st, pool, warm=None):
    """lg_row(PE) -> exp(ACT; 1/4096 pool normalization folded into the
    activation scale, softmax denominator free via accum_out)."""
    lgr_ps = ps.tile([1, K], F32, tag="tiny")
    nc.tensor.matmul(lgr_ps[:], st["prelu"][:], cc.attk, start=True,
                     stop=True)
    if warm:
        warm()
    er_sb = sb.tile([1, K], F16, tag="erow")
    s_sb = sb.tile([1, 1], F32, tag="ssum")
    exp_ins = nc.scalar.activation(er_sb[:], lgr_ps[:], AF.Exp,
                                   scale=float(1.0 / NPOS),
                                   accum_out=s_sb[:])
    pool["exp"] = exp_ins
    st["er"] = er_sb
    st["s"] = s_sb


def _emit_chain_stage3(nc, b, sb, ps, cc, wk_sb, wk_full, wm_sb, beta_sb,
                       invs_sb, st, pool, grouped=False):
    """att broadcast(PE) -> copy(DVE) -> mixing MACs(DVE), plus the
    off-critical-path normalization (epilogue scale + bias).

    The softmax is left UNNORMALIZED — mixing uses raw exp weights and the
    1/sum lands in the epilogue's per-partition activation scale (invs_sb),
    along with the matching bias correction (beta_sb).
    """
    er_sb, s_sb, prelu_sb = st["er"], st["s"], st["prelu"]
    ab_ps = ps.tile([C, K], F32, tag="tiny")
    nc.tensor.matmul(ab_ps[:], cc.ones16, er_sb[:], start=True, stop=True)
    ab_sb = sb.tile([C, K], F32, tag="abc")
    nc.vector.tensor_copy(ab_sb[:], ab_ps[:])

    # Mix expert bank with UNNORMALIZED weights: wm = sum_k e[k] * wk[k].
    # grouped=True (sample 0) emits one tap-group at a time so conv taps
    # 3g..3g+2 unblock early; otherwise one full-width strided op per
    # expert (DVE ops carry ~0.5us fixed overhead — fewer, bigger is
    # better when latency doesn't matter).
    last = None
    if grouped:
        for g in range(GROUPS):
            dst = wm_sb[:, g * GW:(g + 1) * GW]
            m0 = nc.vector.tensor_scalar_mul(dst, wk_sb(g, 0), ab_sb[:, 0:1])
            if last is not None:
                add_dep_helper(m0.ins, last.ins,
                               reason="mix groups strictly in order")
            for k in range(1, K):
                last = nc.vector.scalar_tensor_tensor(
                    dst, wk_sb(g, k), ab_sb[:, k:k + 1], dst,
                    op0=ALU.mult, op1=ALU.add)
    else:
        dst = wm_sb[:, 0:TAPS * NF]
        nc.vector.tensor_scalar_mul(dst, wk_full(0), ab_sb[:, 0:1])
        for k in range(1, K):
            last = nc.vector.scalar_tensor_tensor(
                dst, wk_full(k), ab_sb[:, k:k + 1], dst,
                op0=ALU.mult, op1=ALU.add)

    rec_sb = sb.tile([1, 1], F32, tag="rec")
    nc.vector.reciprocal(rec_sb[:], s_sb[:])
    invs_ps = ps.tile([C, 1], F32, tag="tiny")
    nc.tensor.matmul(invs_ps[:], cc.ones32, rec_sb[:], start=True, stop=True)
    nc.vector.tensor_copy(invs_sb[:], invs_ps[:])
    lgc_ps = ps.tile([K, 1], F32, tag="tiny")
    nc.tensor.matmul(lgc_ps[:], cc.attk, prelu_sb[:], start=True, stop=True)
    ec_sb = sb.tile([K, 1], F16, tag="ecol")
    nc.scalar.activation(ec_sb[:], lgc_ps[:], AF.Exp, scale=float(1.0 / NPOS))
    bm_ps = ps.tile([NF, 1], F32, tag="tiny")
    nc.tensor.matmul(bm_ps[:], cc.biasw, ec_sb[:], start=True, stop=True)
    nc.vector.tensor_scalar(beta_sb[:], bm_ps[:], invs_sb[:], cc.c1,
                            op0=ALU.mult, op1=ALU.add)
    pool["mix_last"] = last
    return pool


def _emit_chain(nc, b, sb, ps, cc, wk_sb, wk_full, wm_sb, beta_sb, invs_sb,
                pool, grouped=False, warm=None):
    """Full routing chain (used monolithically for sample 0; samples 1-3
    interleave the three stages between the previous sample's conv chunks
    so each cross-engine hop hides inside a chunk's duration)."""
    st = _emit_chain_stage1(nc, sb, ps, cc, pool, warm=warm)
    _emit_chain_stage2(nc, sb, ps, cc, st, pool, warm=warm)
    return _emit_chain_stage3(nc, b, sb, ps, cc, wk_sb, wk_full, wm_sb,
                              beta_sb, invs_sb, st, pool, grouped=grouped)


def _emit_conv_chunks(nc, b, convps, xt_sb, wm_sb, beta_sb, invs_sb, y_sb,
                      y_dram, t_lo, t_hi, last_sample=False):
    """9-tap conv chunks [t_lo, t_hi) as shifted fp16 matmuls + fused
    BN/bias/relu epilogue; fp16 output DMA'd out in pieces (sync + gpsimd
    queues, keeping the scalar queue free for epilogues)."""
    xv = xt_sb[:, :NPAD].rearrange("p (h w) -> p h w", w=WP)
    for t in range(t_lo, t_hi):
        pc = convps.tile([NF, ROWS_PER_CHUNK * W], F32, tag="conv")
        for tap in range(TAPS):
            dy, dx = tap // 3, tap % 3
            r0 = ROWS_PER_CHUNK * t + dy
            rhs = xv[:, r0:r0 + ROWS_PER_CHUNK, dx:dx + W]
            nc.tensor.matmul(pc[:], wm_sb[:, NF * tap:NF * (tap + 1)], rhs,
                             start=(tap == 0), stop=(tap == TAPS - 1))
        if last_sample and t == NCHUNK - 1:
            # split the final epilogue so the tail DMA starts sooner
            nc.scalar.activation(y_sb[:, 3584:3840], pc[:, 0:256], AF.Relu,
                                 bias=beta_sb[:], scale=invs_sb[:])
            nc.gpsimd.dma_start(y_dram[b][:, 3584:3840], y_sb[:, 3584:3840])
            nc.scalar.activation(y_sb[:, 3840:4096], pc[:, 256:512], AF.Relu,
                                 bias=beta_sb[:], scale=invs_sb[:])
            nc.sync.dma_start(y_dram[b][:, 3840:], y_sb[:, 3840:])
            continue
        nc.scalar.activation(y_sb[:, 512 * t:512 * (t + 1)], pc[:], AF.Relu,
                             bias=beta_sb[:], scale=invs_sb[:])
        if t == 3:
            nc.sync.dma_start(y_dram[b][:, :2048], y_sb[:, :2048])
        elif t == 5:
            nc.gpsimd.dma_start(y_dram[b][:, 2048:3072], y_sb[:, 2048:3072])
        elif t == 6:
            nc.sync.dma_start(y_dram[b][:, 3072:3584], y_sb[:, 3072:3584])
        elif t == 7:
            nc.gpsimd.dma_start(y_dram[b][:, 3584:3840], y_sb[:, 3584:3840])
            nc.sync.dma_start(y_dram[b][:, 3840:], y_sb[:, 3840:])


def _emit_conv_pass(nc, b, pcs, xt_sb, wm_sb, g):
    """Pass g: taps 3g..3g+2 over chunks 0-3 of sample b. Pipelines the
    conv against the 3-group mixing: pass g only needs mixing group g."""
    xv = xt_sb[:, :NPAD].rearrange("p (h w) -> p h w", w=WP)
    for c in range(len(pcs)):
        for tap in range(3 * g, 3 * g + 3):
            dy, dx = tap // 3, tap % 3
            r0 = ROWS_PER_CHUNK * c + dy
            rhs = xv[:, r0:r0 + ROWS_PER_CHUNK, dx:dx + W]
            nc.tensor.matmul(pcs[c][:], wm_sb[:, NF * tap:NF * (tap + 1)],
                             rhs, start=(tap == 0), stop=(tap == TAPS - 1))


def _emit_pass_epilogues(nc, b, pcs, beta_sb, invs_sb, y_sb, y_dram):
    for c in range(len(pcs)):
        nc.scalar.activation(y_sb[:, 512 * c:512 * (c + 1)], pcs[c][:],
                             AF.Relu, bias=beta_sb[:], scale=invs_sb[:])
    nc.sync.dma_start(y_dram[b][:, :2048], y_sb[:, :2048])


def _build_program():
    nc = bacc.Bacc("TRN2", target_bir_lowering=False, debug=False,
                   num_devices=NCORES)
    xt = nc.dram_tensor("xt", [BPC, C, XCOLS], F16, kind="ExternalInput").ap()
    wk = nc.dram_tensor("wk", [C, GROUPS * WGK], F16,
                        kind="ExternalInput").ap()
    y = nc.dram_tensor("y", [BPC, NF, NPOS], F16, kind="ExternalOutput").ap()

    with tile.TileContext(nc) as tc:
        with (
            tc.tile_pool(name="const", bufs=1) as cpool,
            tc.tile_pool(name="xt", bufs=BPC) as xpool,
            tc.tile_pool(name="wm", bufs=BPC) as wmpool,
            tc.tile_pool(name="work", bufs=4) as sb,
            tc.tile_pool(name="ystage", bufs=2) as ypool,
            tc.tile_pool(name="convps", bufs=5, space="PSUM") as convps,
            tc.tile_pool(name="tinyps", bufs=2, space="PSUM") as ps,
        ):
            xt_sb = [xpool.tile([C, XCOLS], F16, tag="xt", name=f"xt{b}")
                     for b in range(BPC)]
            wk_all = cpool.tile([C, GROUPS * WGK], F16)

            # HBM reads aggregate to only ~255 GB/s, split evenly across
            # ACTIVE rings — so x0's two halves get the bus to themselves
            # first (scalar + gpsimd rings, ~4.4us each; the gpsimd issue
            # goes ahead of the warm-up memsets), and the bank follows on
            # the sync ring as three tap-group transfers dep-chained behind
            # the first half so the scheduler cannot float them into the x0
            # window. Consts ride inside x0's second half.
            xa = nc.scalar.dma_start(xt_sb[0][:, :HALF], xt[0][:, :HALF])
            nc.gpsimd.dma_start(xt_sb[0][:, HALF:], xt[0][:, HALF:])
            dep = xa
            for g in range(GROUPS):
                wd = nc.sync.dma_start(wk_all[:, g * WGK:(g + 1) * WGK],
                                       wk[:, g * WGK:(g + 1) * WGK])
                add_dep_helper(wd.ins, dep.ins,
                               reason="bank groups after x0 first half")
                dep = wd

            # On-device constants: ones rows + zeroed warm-up matmul source
            # (no DMA; memsets queue behind the gpsimd x0 issue).
            ones16_sb = cpool.tile([1, C], F16, tag="ones16")
            nc.gpsimd.memset(ones16_sb[:], 1.0)
            ones32_sb = cpool.tile([1, C], F32, tag="ones32")
            nc.gpsimd.memset(ones32_sb[:], 1.0)
            warm_src = cpool.tile([C, 512], F16, tag="warmsrc")
            nc.gpsimd.memset(warm_src[:], 0.0)
            cc = _Consts(xt_sb[0][:], ones16_sb[:], ones32_sb[:])
            # tensor_scalar requires f32 scalar operands: up-convert c1 once
            c1_32 = cpool.tile([C, 1], F32, tag="c132")
            nc.vector.tensor_copy(c1_32[:], cc.c1)
            cc.c1 = c1_32[:]

            def wk_sb(g, k):
                base = g * WGK + k * GW
                return wk_all[:, base:base + GW]

            def wk_full(k):
                # expert k's full bank as a strided 3D view over the
                # group-major layout: [C, 3 groups (step WGK), 384]
                v = wk_all[:].rearrange("p (g x) -> p g x", x=WGK)
                return v[:, :, k * GW:(k + 1) * GW]

            # Pre-load the ACT spline table set (relu+exp share one set).
            warm_sb = cpool.tile([1, 1], F32, tag="warm")
            nc.scalar.activation(warm_sb[:], ones32_sb[:, 0:1], AF.Exp)

            trash = cpool.tile([C, NPAD], F16, tag="trash")

            wm_sb = [wmpool.tile([C, TAPS * NF], F16, tag="wm",
                                 name=f"wm{b}") for b in range(BPC)]
            beta_sb = [sb.tile([NF, 1], F32, tag="beta", name=f"beta{b}")
                       for b in range(BPC)]
            invs_sb = [sb.tile([NF, 1], F32, tag="invs", name=f"invs{b}")
                       for b in range(BPC)]
            y_sb = [ypool.tile([NF, NPOS], F16, tag="ystage", name=f"yst{b}")
                    for b in range(BPC)]

            # PE warm-up: fine-grained fp16 matmuls on the memset source so
            # the array stays busy (HAM at full clock) through the startup
            # window.
            warm_ps = ps.tile([NF, 512], F32, tag="warmps", bufs=1)

            def pe_warm(n, cols=256, dep=None):
                for _ in range(n):
                    mm = nc.tensor.matmul(warm_ps[:, :cols],
                                          warm_src[:, 0:NF],
                                          warm_src[:, 0:cols], start=True,
                                          stop=True)
                    if dep is not None:
                        add_dep_helper(mm.ins, dep.ins,
                                       reason="hold warm-up for idle window")
                        dep = None

            def emit_next_xt(bn, prev, late=False):
                # Sample bn's input on the GPSIMD + scalar rings, gated so
                # the issue ops can't be scheduled into the previous
                # routing-chain window (a DMA issue op occupies its engine
                # queue for ~0.6us); sample 1 additionally waits out the
                # startup loads (HBM bandwidth).
                da = nc.gpsimd.dma_start(xt_sb[bn][:, :HALF],
                                         xt[bn][:, :HALF])
                db = nc.scalar.dma_start(xt_sb[bn][:, HALF:],
                                         xt[bn][:, HALF:])
                add_dep_helper(da.ins,
                               prev["exp" if late else "pool_a"].ins,
                               reason="stagger input DMA bandwidth")
                add_dep_helper(db.ins,
                               prev["exp" if late else "pool_a"].ins,
                               reason="keep ACT-queue DMA issue after chain")

            pe_warm(WARM1, cols=512)

            chains = [None] * BPC
            chains[0] = _emit_pool(nc, 0, sb, xt_sb[0][:], trash)
            _emit_chain(nc, 0, sb, ps, cc, wk_sb, wk_full, wm_sb[0],
                        beta_sb[0], invs_sb[0], chains[0], grouped=True,
                        warm=lambda: pe_warm(WARM2))
            emit_next_xt(1, chains[0], late=True)
            # warm-ups held (via dep) until the chain frees the PE, filling
            # the mixing-group-0 window at full clock
            pe_warm(WARM3, cols=512, dep=chains[0]["exp"])

            # Per sample b: chunks 0-3 run as three tap-group passes
            # pipelined against the 3-group mixing (pass g needs only
            # group g); the NEXT sample's routing chain is emitted in
            # stages between the passes/chunks so each cross-engine hop
            # hides inside ~2us of conv work, and its mixing groups land
            # just ahead of the next sample's passes.
            for b in range(BPC):
                nb = b + 1
                if nb < BPC:
                    chains[nb] = _emit_pool(nc, nb, sb, xt_sb[nb][:], trash,
                                            act_only=(nb >= BPC - 2))
                    if not chains[nb]["act_only"]:
                        add_dep_helper(chains[nb]["pool_b"].ins,
                                       chains[b]["mix_last"].ins,
                                       reason="keep DVE reduce after prev mix")
                pcs = [convps.tile([NF, ROWS_PER_CHUNK * W], F32, tag="conv",
                                   name=f"b{b}p{c}") for c in range(4)]
                _emit_conv_pass(nc, b, pcs, xt_sb[b][:], wm_sb[b], 0)
                _emit_conv_pass(nc, b, pcs, xt_sb[b][:], wm_sb[b], 1)
                _emit_conv_pass(nc, b, pcs, xt_sb[b][:], wm_sb[b], 2)
                if nb < BPC:
                    st = _emit_chain_stage1(nc, sb, ps, cc, chains[nb])
                _emit_pass_epilogues(nc, b, pcs, beta_sb[b], invs_sb[b],
                                     y_sb[b], y)
                _emit_conv_chunks(nc, b, convps, xt_sb[b][:], wm_sb[b],
                                  beta_sb[b], invs_sb[b], y_sb[b], y, 4, 5)
                if nb < BPC:
                    _emit_chain_stage2(nc, sb, ps, cc, st, chains[nb])
                _emit_conv_chunks(nc, b, convps, xt_sb[b][:], wm_sb[b],
                                  beta_sb[b], invs_sb[b], y_sb[b], y, 5, 6)
                if nb < BPC:
                    _emit_chain_stage3(nc, nb, sb, ps, cc, wk_sb, wk_full,
                                       wm_sb[nb], beta_sb[nb], invs_sb[nb],
                                       st, chains[nb], grouped=True)
                _emit_conv_chunks(nc, b, convps, xt_sb[b][:], wm_sb[b],
                                  beta_sb[b], invs_sb[b], y_sb[b], y, 6, 7)
                if nb < BPC and nb + 1 < BPC:
                    emit_next_xt(nb + 1, chains[nb])
                _emit_conv_chunks(nc, b, convps, xt_sb[b][:], wm_sb[b],
                                  beta_sb[b], invs_sb[b], y_sb[b], y, 7,
                                  NCHUNK, last_sample=(b == BPC - 1))

    nc.compile()
    return nc


_PROGRAM = None


def _get_program():
    global _PROGRAM
    if _PROGRAM is None:
        _PROGRAM = _build_program()
    return _PROGRAM


def _prepare_host_inputs(x, reduction_kernel, attention_kernel, conv_kernels,
                         bias, bn_scale, bn_bias, bn_mean, bn_var):
    f = np.float32
    # Channel-major zero-padded fp16 input [B, C, 66*66], with the fp16
    # routing/epilogue constants appended per sample (each core reads them
    # from ITS first sample's tile).
    xt = np.zeros((B, C, XCOLS), dtype=np.float16)
    xt[:, :, :NPAD] = np.pad(
        x.transpose(0, 3, 1, 2).reshape(B, C, H, W),
        ((0, 0), (0, 0), (1, 1), (1, 1))).reshape(B, C, NPAD)

    inv = (bn_scale / np.sqrt(bn_var + np.float32(1e-5))).astype(f)
    xt[:, :, NPAD:NPAD + 4] = reduction_kernel.astype(np.float16)
    xt[:, 0:4, NPAD + 4:NPAD + 8] = (attention_kernel / f(30.0)).astype(
        np.float16)
    xt[:, 0:4, NPAD + 8:NPAD + 136] = (bias * inv).astype(np.float16)
    xt[:, :, NPAD + 136] = (bn_bias - bn_mean * inv).astype(np.float16)

    # Expert bank fp16, BN folded, tap-GROUP-major: [C, g, k, 3*F] so each
    # mixing group is one contiguous DMA and per-expert full-width views
    # are clean strided APs.
    wkh = (conv_kernels.transpose(0, 3, 1, 2, 4) * inv).astype(f)
    wkh = wkh.reshape(K, C, GROUPS, 3 * NF).transpose(1, 2, 0, 3)
    wkh = np.ascontiguousarray(wkh.reshape(C, GROUPS * WGK),
                               dtype=np.float16)

    in_maps = []
    for cix in range(NCORES):
        in_maps.append({
            "xt": np.ascontiguousarray(xt[cix * BPC:(cix + 1) * BPC]),
            "wk": wkh,
        })
    return in_maps


def kernel(x, reduction_kernel, attention_kernel, conv_kernels, bias, bn_scale,
           bn_bias, bn_mean, bn_var, _trace=False):
    nc = _get_program()
    in_maps = _prepare_host_inputs(
        np.asarray(x, dtype=np.float32), np.asarray(reduction_kernel, np.float32),
        np.asarray(attention_kernel, np.float32),
        np.asarray(conv_kernels, np.float32), np.asarray(bias, np.float32),
        np.asarray(bn_scale, np.float32), np.asarray(bn_bias, np.float32),
        np.asarray(bn_mean, np.float32), np.asarray(bn_var, np.float32))
    if _trace:
        _ensure_ntff_hook()
    res = run_bass_kernel_spmd(nc, in_maps, core_ids=list(range(NCORES)),
                               trace=_trace)
    yt = np.concatenate([res.results[cix]["y"] for cix in range(NCORES)],
                        axis=0)  # [B, F, 4096] fp16
    out = yt.astype(np.float32).reshape(B, NF, H, W).transpose(0, 2, 3, 1)
    out = np.ascontiguousarray(out, dtype=np.float32)
    if _trace:
        return out, res
    return out
